# revision 31
# baseline (speedup 1.0000x reference)
"""Trainium2 Bass kernel for nn_CNNVectorForm (LeNet-style CNN, batch 8192).

Pipeline per core (data-parallel over batch, 1024 images/core):
  conv 5x5 VALID (1->20ch, 28->24)  -> 2x2 maxpool -> fc1(2880->500) + relu
  -> fc2(500->10) + softmax

Device formulation:
  * All activations feature-major [features, batch]; batch rides the free
    dim (nb per tile).  Everything except PSUM accumulation runs in bf16
    (matmul is 1 cycle/row for bf16 same as fp32r, but DMA halves and the
    DVE gets its 2x/4x 16-bit modes).
  * Conv as a Toeplitz matmul.  One [128, 2, nb] gather covers 8 input
    rows x 16 cols (both 12-col output halves), enough for 4 conv output
    rows.  Four row-shifted copies of the Toeplitz matrix [128, 240]
    (zero rows outside the 5-row window) turn each gather into 16 K=128
    matmuls producing [20ch x 6col, nb] per (row, parity) with output
    columns split even/odd so the 2x2 maxpool is partition-aligned.
  * Maxpool reads PSUM directly: the early (even) pair is reduced on the
    Pool engine (gpsimd), the late (odd) pair on the DVE, and the final
    combine is a 4x-mode scalar_tensor_tensor on the DVE.  No scalar
    copies; the Activation engine only does relu / softmax.
  * fc1 weights host-permuted to pooled-feature order; 24 accumulating
    K=120 matmuls per 125-neuron M-tile ride along with the conv loop,
    skewed by 4 blocks, to keep the PE gap-free.
  * conv bias folded into the fc1 bias on the host.
  * fc2 feature-major; softmax via PE transpose of 128-batch slices.
"""

import numpy as np

N, H, W = 8192, 28, 28
COUT, KS = 20, 5
NCORES = 8
NPC = N // NCORES  # images per core
CONV_W_OUT = 24
PH = 12            # pooled rows
FC1_IN, FC1_OUT, FC2_OUT = 2880, 500, 10
MT, MTS = 4, 125   # fc1 M tiles
KB, KBS = 24, 120  # a1 feature blocks (one per (pooled row, column half))
NG = 6             # gathers per batch tile (each covers 4 conv rows)

_cache = {}


def _build(npc, nb):
    from contextlib import ExitStack

    import concourse.tile as tile
    from concourse import bacc, mybir

    f32 = mybir.dt.float32
    bf16 = mybir.dt.bfloat16
    nbt = npc // nb

    nc = bacc.Bacc(
        "TRN2",
        target_bir_lowering=False,
        debug=False,
        enable_asserts=False,
        num_devices=NCORES,
    )

    # host-gathered input: xh[gi, d*16+jjp, jb, b] = x[b, (4gi+d)*28+12jb+jjp]
    xh_d = nc.dram_tensor(
        "xh", [NG, 128, 2, npc], bf16, kind="ExternalInput"
    ).ap()
    # 4 row-shifted Toeplitz mats, [128, s*240 + eo*120 + c*6 + q]
    tm_d = nc.dram_tensor("tm", [128, 4 * 240], bf16, kind="ExternalInput").ap()
    w1_d = nc.dram_tensor(
        "w1", [KBS, KB * FC1_OUT], bf16, kind="ExternalInput"
    ).ap()
    b1_d = nc.dram_tensor("b1", [MTS, MT + 1], f32, kind="ExternalInput").ap()
    w2_d = nc.dram_tensor("w2", [MTS, MT * FC2_OUT], bf16, kind="ExternalInput").ap()
    o_d = nc.dram_tensor("out", [npc, FC2_OUT], f32, kind="ExternalOutput").ap()

    AL = mybir.AluOpType

    with tile.TileContext(nc) as tc, ExitStack() as ctx:
        const = ctx.enter_context(tc.tile_pool(name="const", bufs=1))
        w1pool = ctx.enter_context(tc.tile_pool(name="w1", bufs=6))
        gpool = ctx.enter_context(tc.tile_pool(name="gather", bufs=3))
        a1pool = ctx.enter_context(tc.tile_pool(name="a1", bufs=8))
        mpool = ctx.enter_context(tc.tile_pool(name="ptmp", bufs=3))
        a2pool = ctx.enter_context(tc.tile_pool(name="a2", bufs=2 * MT))
        smpool = ctx.enter_context(tc.tile_pool(name="softmax", bufs=4))
        cpsum = ctx.enter_context(tc.tile_pool(name="cpsum", bufs=4, space="PSUM"))
        fpsum = ctx.enter_context(tc.tile_pool(name="fpsum", bufs=4, space="PSUM"))

        from concourse.masks import make_identity

        # Toeplitz matrix in two halves (shifts 0-1 / 2-3) so the first conv
        # block only waits on half the transfer; gathers split per column
        # half for the same reason (subtile deps let kb=0 start on jb=0).
        tmt = const.tile([128, 4 * 240], bf16)
        nc.sync.dma_start(tmt[:, 0:480], tm_d[:, 0:480])

        gtiles = {}

        def issue_gather(bt, gi):
            if (bt, gi) in gtiles:
                return
            g = gpool.tile([128, 2, nb], bf16, tag="g", name=f"g{bt}_{gi}")
            for jb in range(2):
                nc.sync.dma_start(
                    g[:, jb, :], xh_d[gi, :, jb, bt * nb : (bt + 1) * nb]
                )
            gtiles[(bt, gi)] = g

        issue_gather(0, 0)
        nc.sync.dma_start(tmt[:, 480:960], tm_d[:, 480:960])
        issue_gather(0, 1)

        issue_gather(0, 2)

        # fc1 weights: 8 resident groups of 3 blocks, all on the SYNC
        # queue right after the first two gathers.  The DMA queue is FIFO,
        # so the 2.9MB weight stream physically serializes behind the
        # startup-critical Toeplitz + gather transfers without any
        # semaphore pacing (engine program order does NOT hold DMA issues
        # back, but per-queue transfer order does).
        WG = 3
        w1g = []
        for gidx in range(KB // WG):
            wt = w1pool.tile([KBS, WG * FC1_OUT], bf16, tag="w1", bufs=8,
                             name=f"w1g{gidx}")
            nc.sync.dma_start(
                wt[:], w1_d[:, gidx * WG * FC1_OUT : (gidx + 1) * WG * FC1_OUT]
            )
            w1g.append(wt)

        b1t = const.tile([MTS, MT + 1], f32)
        nc.scalar.dma_start(b1t[:], b1_d[:])
        w2t = const.tile([MTS, MT * FC2_OUT], bf16)
        nc.scalar.dma_start(w2t[:], w2_d[:])
        ident = const.tile([FC2_OUT, FC2_OUT], f32)
        make_identity(nc, ident[:])

        # PE p-state warmup: ~3us of throwaway matmuls while the input
        # DMAs land, so the real stream starts at full clock.
        # PE p-state warmup: fine-grained throwaway matmuls keep the PE
        # continuously busy from as early as possible until the first
        # gather lands, so the clock ramp completes without a reset and
        # the real stream tacks on with at most one small matmul of delay.
        wz = const.tile([128, nb], bf16)
        nc.gpsimd.memset(wz[:], 0)
        wm = min(128, nb)
        wr = min(128, nb)
        for wi in range(28):
            wp = cpsum.tile([128, nb], f32, tag="cps", name=f"warm{wi}")
            nc.tensor.matmul(
                wp[0:wm, 0:wr], wz[:, 0:wm], wz[:, 0:wr], start=True, stop=True
            )

        def w1_slice(j, mt):
            off = (j % WG) * FC1_OUT + mt * MTS
            return w1g[j // WG][:, off : off + MTS]

        for bt in range(nbt):
            b0 = bt * nb
            a1 = [None] * KB

            if bt > 0:
                issue_gather(bt, 0)
                issue_gather(bt, 1)

            # fc1 accumulators for all 4 M-tiles ride along with the conv
            # loop, skewed by 4 blocks (one gather's worth).  conv and fc1
            # matmuls are interleaved pairwise so each conv PSUM tile lands
            # early in the period, giving the maxpool chain slack to free
            # the single-buffered PSUM ring before the next block needs it.
            fp = [
                fpsum.tile([MTS, nb], f32, tag="fps", name=f"fp{bt}_{mt}")
                for mt in range(MT)
            ]
            SKEW = 2
            for kb in range(KB + SKEW):
                j = kb - SKEW
                conv = kb < KB
                if conv:
                    gi, sub = kb // 4, kb % 4
                    if sub == 0 and gi + 2 < NG:
                        issue_gather(bt, gi + 2)
                    ipr, jb = sub // 2, sub % 2
                    g = gtiles[(bt, gi)]
                    ps = [
                        cpsum.tile([KBS, nb], f32, tag="cps", name=f"cps{i}")
                        for i in range(4)
                    ]
                for i in range(4):
                    if conv:
                        eo, dr = i // 2, i % 2
                        s = 2 * ipr + dr
                        nc.tensor.matmul(
                            ps[i][:],
                            tmt[:, s * 240 + eo * 120 : s * 240 + (eo + 1) * 120],
                            g[:, jb, :],
                            start=True,
                            stop=True,
                        )
                    if j >= 0:
                        nc.tensor.matmul(
                            fp[i][:],
                            w1_slice(j, i),
                            a1[j][:],
                            start=(j == 0),
                            stop=(j == KB - 1),
                        )
                if not conv:
                    continue
                # 2x2 maxpool.  HW: vector ops may read at most one PSUM
                # operand, and GPSIMD supports neither PSUM nor max.  The
                # Activation engine copies one tile of each vertical pair
                # to SBUF, the DVE does both single-PSUM maxes (bf16 out)
                # plus the all-bf16 horizontal combine (2x mode).
                c0 = mpool.tile([KBS, nb], f32, tag="c0")
                nc.scalar.copy(c0[:], ps[0][:])
                me = mpool.tile([KBS, nb], bf16, tag="me")
                nc.vector.tensor_max(me[:], c0[:], ps[1][:])
                c2 = mpool.tile([KBS, nb], f32, tag="c2")
                nc.scalar.copy(c2[:], ps[2][:])
                mo = mpool.tile([KBS, nb], bf16, tag="mo")
                nc.vector.tensor_max(mo[:], c2[:], ps[3][:])
                ab = a1pool.tile([KBS, nb], bf16, tag="a1")
                nc.vector.tensor_max(ab[:], me[:], mo[:])
                a1[kb] = ab

            if bt + 1 < nbt:
                issue_gather(bt + 1, 0)
                issue_gather(bt + 1, 1)

            a2t = [None] * MT
            for mt in range(MT):
                a2 = a2pool.tile([MTS, nb], bf16, tag="a2")
                nc.scalar.activation(
                    a2[:],
                    fp[mt][:],
                    mybir.ActivationFunctionType.Relu,
                    bias=b1t[:, mt : mt + 1],
                )
                a2t[mt] = a2

            # fc2 feature-major: weights stationary, batch streams; softmax
            # needs batch on partitions, so PE-transpose 128-wide slices.
            p2f = fpsum.tile([FC2_OUT, nb], f32, tag="fps", name=f"p2f_{bt}")
            for mt in range(MT):
                nc.tensor.matmul(
                    p2f[:],
                    w2t[:, mt * FC2_OUT : (mt + 1) * FC2_OUT],
                    a2t[mt][:],
                    start=(mt == 0),
                    stop=(mt == MT - 1),
                )
            s2 = smpool.tile([FC2_OUT, nb], f32, tag="s2")
            nc.scalar.activation(
                s2[:], p2f[:], mybir.ActivationFunctionType.Identity,
                bias=b1t[0:FC2_OUT, MT : MT + 1],
            )
            sub = min(128, nb)
            nsub = nb // sub
            stg = smpool.tile([sub, nsub, FC2_OUT], f32, tag="stg",
                              name=f"stg{bt}")
            for s in range(nsub):
                tp = fpsum.tile([sub, FC2_OUT], f32, tag="fps",
                                name=f"tp_{bt}_{s}")
                nc.tensor.transpose(
                    tp[:], s2[:, s * sub : (s + 1) * sub], ident[:]
                )
                e = smpool.tile([sub, FC2_OUT], f32, tag="e")
                ssum = smpool.tile([sub, 1], f32, tag="ss")
                nc.scalar.activation(
                    e[:], tp[:], mybir.ActivationFunctionType.Exp,
                    accum_out=ssum[:],
                )
                rinv = smpool.tile([sub, 1], f32, tag="ri")
                nc.vector.reciprocal(rinv[:], ssum[:])
                nc.vector.tensor_scalar_mul(stg[:, s, :], e[:], rinv[:])
            nc.scalar.dma_start(
                o_d[b0 : b0 + nb, :].rearrange("(s p) f -> p s f", p=sub),
                stg[:],
            )

    nc.compile()
    return nc


def _prep_weights(conv_w, conv_b, fc1_w, fc1_b, fc2_w, fc2_b):
    import ml_dtypes

    bf = ml_dtypes.bfloat16
    conv_w = np.asarray(conv_w, np.float32).reshape(COUT, KS, KS)
    conv_b = np.asarray(conv_b, np.float32)
    fc1_w = np.asarray(fc1_w, np.float32)
    fc1_b = np.asarray(fc1_b, np.float32)
    fc2_w = np.asarray(fc2_w, np.float32)
    fc2_b = np.asarray(fc2_b, np.float32)

    # Toeplitz mats [128, 4*240]: row = d*16 + jjp (input row d in the
    # 8-row gather, col jjp in the 16-wide block); col = s*240 + m with
    # m = eo*120 + c*6 + q for output col jj = 2q + eo, conv row = gather
    # row s (shift), taps at d = s + di, jjp = jj + dj.
    T = np.zeros((128, 4 * 240), np.float32)
    for s in range(4):
        for m in range(240):
            eo, c, q = m // 120, (m % 120) // 6, m % 6
            jj = 2 * q + eo
            for di in range(KS):
                for dj in range(KS):
                    T[(s + di) * 16 + jj + dj, s * 240 + m] = conv_w[c, di, dj]

    # fc1 weights permuted to our pooled-feature order:
    # block kb = ip*2 + jb, within-block m = c*6 + q
    # -> original flat feature c*144 + ip*12 + jb*6 + q
    kbv = np.arange(KB)
    ipv, jbv = kbv // 2, kbv % 2
    ml = np.arange(KBS)
    cv, qv = ml // 6, ml % 6
    fidx = cv[None, :] * 144 + ipv[:, None] * 12 + jbv[:, None] * 6 + qv[None, :]
    w1 = fc1_w.T[fidx.reshape(-1)].reshape(KB, KBS, FC1_OUT)
    # flat j-major layout [120, 24*500]; the device streams column ranges
    w1 = np.ascontiguousarray(w1.transpose(1, 0, 2)).reshape(KBS, KB * FC1_OUT)

    # conv bias folded into fc1 bias (pool-max commutes with per-channel const)
    cb_vec = np.repeat(conv_b, 144)
    b1p = fc1_b + fc1_w @ cb_vec
    b1 = np.zeros((MTS, MT + 1), np.float32)
    b1[:, :MT] = b1p.reshape(MT, MTS).T
    b1[:FC2_OUT, MT] = fc2_b

    w2 = np.ascontiguousarray(
        fc2_w.T.reshape(MT, MTS, FC2_OUT).transpose(1, 0, 2)
    ).reshape(MTS, MT * FC2_OUT)
    return T.astype(bf), w1.astype(bf), b1, w2.astype(bf)


# gather pixel indices: idx[gi, d*16+jjp, jb] = (4*gi+d)*28 + 12*jb + jjp
_IDX = np.zeros((NG, 128, 2), np.int64)
for _gi in range(NG):
    for _d in range(8):
        for _jjp in range(16):
            for _jb in range(2):
                _IDX[_gi, _d * 16 + _jjp, _jb] = (4 * _gi + _d) * W + 12 * _jb + _jjp


def _prep_x(x_core):
    """x_core [784, npc] pixel-major bf16 -> xh [NG, 128, 2, npc]."""
    return np.ascontiguousarray(x_core[_IDX.reshape(-1)].reshape(
        NG, 128, 2, x_core.shape[1]))


def _run(inputs, npc=NPC, nb=512, trace=False):
    import ml_dtypes
    from concourse import bass_utils

    key = (npc, nb)
    if key not in _cache:
        _cache[key] = _build(npc, nb)
    nc = _cache[key]

    T, w1, b1, w2 = _prep_weights(
        inputs["conv_w"], inputs["conv_b"], inputs["fc1_w"],
        inputs["fc1_b"], inputs["fc2_w"], inputs["fc2_b"],
    )
    x = np.asarray(inputs["x"], np.float32).reshape(-1, H * W)
    n_total = x.shape[0]
    assert n_total == NCORES * npc
    xs = x.reshape(NCORES, npc, H * W).transpose(0, 2, 1).astype(ml_dtypes.bfloat16)

    in_maps = [
        {"xh": _prep_x(xs[i]), "tm": T, "w1": w1, "b1": b1, "w2": w2}
        for i in range(NCORES)
    ]
    res = bass_utils.run_bass_kernel_spmd(
        nc, in_maps, core_ids=list(range(NCORES)), trace=trace
    )
    out = np.concatenate([res.results[i]["out"] for i in range(NCORES)], axis=0)
    return out, res


def kernel(**inputs):
    out, _ = _run(inputs)
    return out


# revision 32
# speedup vs baseline: 1.0064x; 1.0064x over previous
"""Trainium2 Bass kernel for nn_CNNVectorForm (LeNet-style CNN, batch 8192).

Pipeline per core (data-parallel over batch, 1024 images/core):
  conv 5x5 VALID (1->20ch, 28->24)  -> 2x2 maxpool -> fc1(2880->500) + relu
  -> fc2(500->10) + softmax

Device formulation:
  * All activations feature-major [features, batch]; batch rides the free
    dim (nb per tile).  Everything except PSUM accumulation runs in bf16
    (matmul is 1 cycle/row for bf16 same as fp32r, but DMA halves and the
    DVE gets its 2x/4x 16-bit modes).
  * Conv as a Toeplitz matmul.  One [128, 2, nb] gather covers 8 input
    rows x 16 cols (both 12-col output halves), enough for 4 conv output
    rows.  Four row-shifted copies of the Toeplitz matrix [128, 240]
    (zero rows outside the 5-row window) turn each gather into 16 K=128
    matmuls producing [20ch x 6col, nb] per (row, parity) with output
    columns split even/odd so the 2x2 maxpool is partition-aligned.
  * Maxpool reads PSUM directly: the early (even) pair is reduced on the
    Pool engine (gpsimd), the late (odd) pair on the DVE, and the final
    combine is a 4x-mode scalar_tensor_tensor on the DVE.  No scalar
    copies; the Activation engine only does relu / softmax.
  * fc1 weights host-permuted to pooled-feature order; 24 accumulating
    K=120 matmuls per 125-neuron M-tile ride along with the conv loop,
    skewed by 4 blocks, to keep the PE gap-free.
  * conv bias folded into the fc1 bias on the host.
  * fc2 feature-major; softmax via PE transpose of 128-batch slices.
"""

import numpy as np

N, H, W = 8192, 28, 28
COUT, KS = 20, 5
NCORES = 8
NPC = N // NCORES  # images per core
CONV_W_OUT = 24
PH = 12            # pooled rows
FC1_IN, FC1_OUT, FC2_OUT = 2880, 500, 10
MT, MTS = 4, 125   # fc1 M tiles
KB, KBS = 24, 120  # a1 feature blocks (one per (pooled row, column half))
NG = 6             # gathers per batch tile (each covers 4 conv rows)

_cache = {}


def _build(npc, nb):
    from contextlib import ExitStack

    import concourse.tile as tile
    from concourse import bacc, mybir

    f32 = mybir.dt.float32
    bf16 = mybir.dt.bfloat16
    nbt = npc // nb

    nc = bacc.Bacc(
        "TRN2",
        target_bir_lowering=False,
        debug=False,
        enable_asserts=False,
        num_devices=NCORES,
    )

    # host-gathered input: xh[gi, d*16+jjp, jb, b] = x[b, (4gi+d)*28+12jb+jjp]
    xh_d = nc.dram_tensor(
        "xh", [NG, 128, 2, npc], bf16, kind="ExternalInput"
    ).ap()
    # 4 row-shifted Toeplitz mats, [128, s*240 + eo*120 + c*6 + q]
    tm_d = nc.dram_tensor("tm", [128, 4 * 240], bf16, kind="ExternalInput").ap()
    w1_d = nc.dram_tensor(
        "w1", [KBS, KB * FC1_OUT], bf16, kind="ExternalInput"
    ).ap()
    b1_d = nc.dram_tensor("b1", [MTS, MT + 1], f32, kind="ExternalInput").ap()
    w2_d = nc.dram_tensor("w2", [MTS, MT * FC2_OUT], bf16, kind="ExternalInput").ap()
    o_d = nc.dram_tensor("out", [npc, FC2_OUT], f32, kind="ExternalOutput").ap()

    AL = mybir.AluOpType

    with tile.TileContext(nc) as tc, ExitStack() as ctx:
        const = ctx.enter_context(tc.tile_pool(name="const", bufs=1))
        w1pool = ctx.enter_context(tc.tile_pool(name="w1", bufs=6))
        gpool = ctx.enter_context(tc.tile_pool(name="gather", bufs=3))
        a1pool = ctx.enter_context(tc.tile_pool(name="a1", bufs=8))
        mpool = ctx.enter_context(tc.tile_pool(name="ptmp", bufs=3))
        a2pool = ctx.enter_context(tc.tile_pool(name="a2", bufs=2 * MT))
        smpool = ctx.enter_context(tc.tile_pool(name="softmax", bufs=4))
        cpsum = ctx.enter_context(tc.tile_pool(name="cpsum", bufs=4, space="PSUM"))
        fpsum = ctx.enter_context(tc.tile_pool(name="fpsum", bufs=4, space="PSUM"))

        from concourse.masks import make_identity

        # Toeplitz matrix in two halves (shifts 0-1 / 2-3) so the first conv
        # block only waits on half the transfer; gathers split per column
        # half for the same reason (subtile deps let kb=0 start on jb=0).
        tmt = const.tile([128, 4 * 240], bf16)
        nc.sync.dma_start(tmt[:, 0:480], tm_d[:, 0:480])

        gtiles = {}

        def issue_gather(bt, gi):
            if (bt, gi) in gtiles:
                return
            g = gpool.tile([128, 2, nb], bf16, tag="g", name=f"g{bt}_{gi}")
            for jb in range(2):
                nc.sync.dma_start(
                    g[:, jb, :], xh_d[gi, :, jb, bt * nb : (bt + 1) * nb]
                )
            gtiles[(bt, gi)] = g

        issue_gather(0, 0)
        nc.sync.dma_start(tmt[:, 480:960], tm_d[:, 480:960])
        issue_gather(0, 1)

        # fc1 weights: 8 resident groups of 3 blocks, all on the SYNC
        # queue interleaved with the startup gathers.  The DMA queue is
        # FIFO, so the 2.9MB weight stream physically serializes behind
        # the startup-critical Toeplitz + gather transfers without any
        # semaphore pacing (engine program order does NOT hold DMA issues
        # back, but per-queue transfer order does).  Group 0 goes before
        # gather 2 (fc1 j=0 starts at kb=SKEW; gather 2 isn't read until
        # kb=8).
        WG = 3
        w1g = []

        def issue_w1(gidx):
            wt = w1pool.tile([KBS, WG * FC1_OUT], bf16, tag="w1", bufs=8,
                             name=f"w1g{gidx}")
            nc.sync.dma_start(
                wt[:], w1_d[:, gidx * WG * FC1_OUT : (gidx + 1) * WG * FC1_OUT]
            )
            w1g.append(wt)

        issue_w1(0)
        issue_gather(0, 2)
        for gidx in range(1, KB // WG):
            issue_w1(gidx)

        b1t = const.tile([MTS, MT + 1], f32)
        nc.scalar.dma_start(b1t[:], b1_d[:])
        w2t = const.tile([MTS, MT * FC2_OUT], bf16)
        nc.scalar.dma_start(w2t[:], w2_d[:])
        ident = const.tile([FC2_OUT, FC2_OUT], f32)
        make_identity(nc, ident[:])

        # PE p-state warmup: ~3us of throwaway matmuls while the input
        # DMAs land, so the real stream starts at full clock.
        # PE p-state warmup: fine-grained throwaway matmuls keep the PE
        # continuously busy from as early as possible until the first
        # gather lands, so the clock ramp completes without a reset and
        # the real stream tacks on with at most one small matmul of delay.
        wz = const.tile([128, nb], bf16)
        nc.gpsimd.memset(wz[:], 0)
        wm = min(128, nb)
        wr = min(128, nb)
        for wi in range(28):
            wp = cpsum.tile([128, nb], f32, tag="cps", name=f"warm{wi}")
            nc.tensor.matmul(
                wp[0:wm, 0:wr], wz[:, 0:wm], wz[:, 0:wr], start=True, stop=True
            )

        def w1_slice(j, mt):
            off = (j % WG) * FC1_OUT + mt * MTS
            return w1g[j // WG][:, off : off + MTS]

        for bt in range(nbt):
            b0 = bt * nb
            a1 = [None] * KB

            if bt > 0:
                issue_gather(bt, 0)
                issue_gather(bt, 1)

            # fc1 accumulators for all 4 M-tiles ride along with the conv
            # loop, skewed by 4 blocks (one gather's worth).  conv and fc1
            # matmuls are interleaved pairwise so each conv PSUM tile lands
            # early in the period, giving the maxpool chain slack to free
            # the single-buffered PSUM ring before the next block needs it.
            fp = [
                fpsum.tile([MTS, nb], f32, tag="fps", name=f"fp{bt}_{mt}")
                for mt in range(MT)
            ]
            SKEW = 2
            for kb in range(KB + SKEW):
                j = kb - SKEW
                conv = kb < KB
                if conv:
                    gi, sub = kb // 4, kb % 4
                    if sub == 0 and gi + 2 < NG:
                        issue_gather(bt, gi + 2)
                    ipr, jb = sub // 2, sub % 2
                    g = gtiles[(bt, gi)]
                    ps = [
                        cpsum.tile([KBS, nb], f32, tag="cps", name=f"cps{i}")
                        for i in range(4)
                    ]
                for i in range(4):
                    if conv:
                        eo, dr = i // 2, i % 2
                        s = 2 * ipr + dr
                        nc.tensor.matmul(
                            ps[i][:],
                            tmt[:, s * 240 + eo * 120 : s * 240 + (eo + 1) * 120],
                            g[:, jb, :],
                            start=True,
                            stop=True,
                        )
                    if j >= 0:
                        nc.tensor.matmul(
                            fp[i][:],
                            w1_slice(j, i),
                            a1[j][:],
                            start=(j == 0),
                            stop=(j == KB - 1),
                        )
                if not conv:
                    continue
                # 2x2 maxpool.  HW: vector ops may read at most one PSUM
                # operand, and GPSIMD supports neither PSUM nor max.  The
                # Activation engine copies one tile of each vertical pair
                # to SBUF, the DVE does both single-PSUM maxes (bf16 out)
                # plus the all-bf16 horizontal combine (2x mode).
                c0 = mpool.tile([KBS, nb], f32, tag="c0")
                nc.scalar.copy(c0[:], ps[0][:])
                me = mpool.tile([KBS, nb], bf16, tag="me")
                nc.vector.tensor_max(me[:], c0[:], ps[1][:])
                c2 = mpool.tile([KBS, nb], f32, tag="c2")
                nc.scalar.copy(c2[:], ps[2][:])
                mo = mpool.tile([KBS, nb], bf16, tag="mo")
                nc.vector.tensor_max(mo[:], c2[:], ps[3][:])
                ab = a1pool.tile([KBS, nb], bf16, tag="a1")
                nc.vector.tensor_max(ab[:], me[:], mo[:])
                a1[kb] = ab

            if bt + 1 < nbt:
                issue_gather(bt + 1, 0)
                issue_gather(bt + 1, 1)

            a2t = [None] * MT
            for mt in range(MT):
                a2 = a2pool.tile([MTS, nb], bf16, tag="a2")
                nc.scalar.activation(
                    a2[:],
                    fp[mt][:],
                    mybir.ActivationFunctionType.Relu,
                    bias=b1t[:, mt : mt + 1],
                )
                a2t[mt] = a2

            # fc2 feature-major: weights stationary, batch streams; softmax
            # needs batch on partitions, so PE-transpose 128-wide slices.
            p2f = fpsum.tile([FC2_OUT, nb], f32, tag="fps", name=f"p2f_{bt}")
            for mt in range(MT):
                nc.tensor.matmul(
                    p2f[:],
                    w2t[:, mt * FC2_OUT : (mt + 1) * FC2_OUT],
                    a2t[mt][:],
                    start=(mt == 0),
                    stop=(mt == MT - 1),
                )
            s2 = smpool.tile([FC2_OUT, nb], f32, tag="s2")
            nc.scalar.activation(
                s2[:], p2f[:], mybir.ActivationFunctionType.Identity,
                bias=b1t[0:FC2_OUT, MT : MT + 1],
            )
            sub = min(128, nb)
            nsub = nb // sub
            stg = smpool.tile([sub, nsub, FC2_OUT], f32, tag="stg",
                              name=f"stg{bt}")
            for s in range(nsub):
                tp = fpsum.tile([sub, FC2_OUT], f32, tag="fps",
                                name=f"tp_{bt}_{s}")
                nc.tensor.transpose(
                    tp[:], s2[:, s * sub : (s + 1) * sub], ident[:]
                )
                e = smpool.tile([sub, FC2_OUT], f32, tag="e")
                ssum = smpool.tile([sub, 1], f32, tag="ss")
                nc.scalar.activation(
                    e[:], tp[:], mybir.ActivationFunctionType.Exp,
                    accum_out=ssum[:],
                )
                rinv = smpool.tile([sub, 1], f32, tag="ri")
                nc.vector.reciprocal(rinv[:], ssum[:])
                nc.vector.tensor_scalar_mul(stg[:, s, :], e[:], rinv[:])
            nc.scalar.dma_start(
                o_d[b0 : b0 + nb, :].rearrange("(s p) f -> p s f", p=sub),
                stg[:],
            )

    nc.compile()
    return nc


def _prep_weights(conv_w, conv_b, fc1_w, fc1_b, fc2_w, fc2_b):
    import ml_dtypes

    bf = ml_dtypes.bfloat16
    conv_w = np.asarray(conv_w, np.float32).reshape(COUT, KS, KS)
    conv_b = np.asarray(conv_b, np.float32)
    fc1_w = np.asarray(fc1_w, np.float32)
    fc1_b = np.asarray(fc1_b, np.float32)
    fc2_w = np.asarray(fc2_w, np.float32)
    fc2_b = np.asarray(fc2_b, np.float32)

    # Toeplitz mats [128, 4*240]: row = d*16 + jjp (input row d in the
    # 8-row gather, col jjp in the 16-wide block); col = s*240 + m with
    # m = eo*120 + c*6 + q for output col jj = 2q + eo, conv row = gather
    # row s (shift), taps at d = s + di, jjp = jj + dj.
    T = np.zeros((128, 4 * 240), np.float32)
    for s in range(4):
        for m in range(240):
            eo, c, q = m // 120, (m % 120) // 6, m % 6
            jj = 2 * q + eo
            for di in range(KS):
                for dj in range(KS):
                    T[(s + di) * 16 + jj + dj, s * 240 + m] = conv_w[c, di, dj]

    # fc1 weights permuted to our pooled-feature order:
    # block kb = ip*2 + jb, within-block m = c*6 + q
    # -> original flat feature c*144 + ip*12 + jb*6 + q
    kbv = np.arange(KB)
    ipv, jbv = kbv // 2, kbv % 2
    ml = np.arange(KBS)
    cv, qv = ml // 6, ml % 6
    fidx = cv[None, :] * 144 + ipv[:, None] * 12 + jbv[:, None] * 6 + qv[None, :]
    w1 = fc1_w.T[fidx.reshape(-1)].reshape(KB, KBS, FC1_OUT)
    # flat j-major layout [120, 24*500]; the device streams column ranges
    w1 = np.ascontiguousarray(w1.transpose(1, 0, 2)).reshape(KBS, KB * FC1_OUT)

    # conv bias folded into fc1 bias (pool-max commutes with per-channel const)
    cb_vec = np.repeat(conv_b, 144)
    b1p = fc1_b + fc1_w @ cb_vec
    b1 = np.zeros((MTS, MT + 1), np.float32)
    b1[:, :MT] = b1p.reshape(MT, MTS).T
    b1[:FC2_OUT, MT] = fc2_b

    w2 = np.ascontiguousarray(
        fc2_w.T.reshape(MT, MTS, FC2_OUT).transpose(1, 0, 2)
    ).reshape(MTS, MT * FC2_OUT)
    return T.astype(bf), w1.astype(bf), b1, w2.astype(bf)


# gather pixel indices: idx[gi, d*16+jjp, jb] = (4*gi+d)*28 + 12*jb + jjp
_IDX = np.zeros((NG, 128, 2), np.int64)
for _gi in range(NG):
    for _d in range(8):
        for _jjp in range(16):
            for _jb in range(2):
                _IDX[_gi, _d * 16 + _jjp, _jb] = (4 * _gi + _d) * W + 12 * _jb + _jjp


def _prep_x(x_core):
    """x_core [784, npc] pixel-major bf16 -> xh [NG, 128, 2, npc]."""
    return np.ascontiguousarray(x_core[_IDX.reshape(-1)].reshape(
        NG, 128, 2, x_core.shape[1]))


def _run(inputs, npc=NPC, nb=512, trace=False):
    import ml_dtypes
    from concourse import bass_utils

    key = (npc, nb)
    if key not in _cache:
        _cache[key] = _build(npc, nb)
    nc = _cache[key]

    T, w1, b1, w2 = _prep_weights(
        inputs["conv_w"], inputs["conv_b"], inputs["fc1_w"],
        inputs["fc1_b"], inputs["fc2_w"], inputs["fc2_b"],
    )
    x = np.asarray(inputs["x"], np.float32).reshape(-1, H * W)
    n_total = x.shape[0]
    assert n_total == NCORES * npc
    xs = x.reshape(NCORES, npc, H * W).transpose(0, 2, 1).astype(ml_dtypes.bfloat16)

    in_maps = [
        {"xh": _prep_x(xs[i]), "tm": T, "w1": w1, "b1": b1, "w2": w2}
        for i in range(NCORES)
    ]
    res = bass_utils.run_bass_kernel_spmd(
        nc, in_maps, core_ids=list(range(NCORES)), trace=trace
    )
    out = np.concatenate([res.results[i]["out"] for i in range(NCORES)], axis=0)
    return out, res


def kernel(**inputs):
    out, _ = _run(inputs)
    return out
